# revision 2
# baseline (speedup 1.0000x reference)
"""MFVI constituency kernel for trn2 (8 NeuronCores, batch-parallel).

Math (per batch b, row i, with S=128):
    q_{t+1}[i,j] = s_span[i,j] + mask[i,j] * sum_k W[j,k] * P_i[j,k] * qz_t[i,k]
where
    P_i[j,k]  = s_pair[b,i,j,k]
    W[j,k]    = (mask[j,k] | mask[k,j]) * [j != k]        (symmetric)
    qz_t[i,k] = sigmoid(q_t[i,k]) * [k != i]
    output    = sigmoid(q_3)

Each (b,i) row evolves independently with its own 128x128 matrix, so the
kernel streams s_pair once (fp32->fp16 cast in the DMA), masks by W on the
DVE, DMA-xbar-transposes each tile so k lands on partitions, and runs the
3 iterations as N=1 matvecs on the TensorEngine.  Matvec outputs for all
128 rows of a batch land as columns of a single PSUM tile, giving a
batched [128,128] epilogue whose transposed layout is exactly the rhs
layout the next iteration needs.
"""

import numpy as np

import concourse.bacc as bacc
import concourse.tile as tile
from concourse import mybir
from concourse.bass_utils import run_bass_kernel_spmd

B, S = 16, 128
N_CORES = 8
B_SH = B // N_CORES  # batches per core
GRP = 16             # i-tiles per load slab
MAX_ITER = 3

f32 = mybir.dt.float32
f16 = mybir.dt.float16
u8 = mybir.dt.uint8

_compiled_nc = None


def build_nc():
    nc = bacc.Bacc("TRN2", target_bir_lowering=False, debug=False,
                   num_devices=N_CORES)

    sspan_d = nc.dram_tensor("s_span", [B_SH, S, S], f32, kind="ExternalInput")
    spair_d = nc.dram_tensor("s_pair", [B_SH, S, S, S], f32, kind="ExternalInput")
    mask_d = nc.dram_tensor("mask", [B_SH, S, S], u8, kind="ExternalInput")
    ident_d = nc.dram_tensor("ident", [S, S], f32, kind="ExternalInput")
    offd32_d = nc.dram_tensor("offd32", [S, S], f32, kind="ExternalInput")
    offd16_d = nc.dram_tensor("offd16", [S, S], f16, kind="ExternalInput")
    out_d = nc.dram_tensor("out", [B_SH, S, S], f32, kind="ExternalOutput")

    with tile.TileContext(nc) as tc:
        with (
            tc.tile_pool(name="consts", bufs=1) as cpool,
            tc.tile_pool(name="prep", bufs=1) as ppool,
            tc.tile_pool(name="spt", bufs=B_SH * S) as spt_pool,
            tc.tile_pool(name="nat", bufs=3) as nat_pool,
            tc.tile_pool(name="msk", bufs=3) as msk_pool,
            tc.tile_pool(name="qz", bufs=4) as qz_pool,
            tc.tile_pool(name="work", bufs=4) as wpool,
            tc.tile_pool(name="ps", bufs=4, space="PSUM") as ps_pool,
            tc.tile_pool(name="pst", bufs=2, space="PSUM") as pst_pool,
        ):
            ident = cpool.tile([S, S], f32, tag="ident")
            nc.sync.dma_start(ident[:], ident_d[:])
            offd32 = cpool.tile([S, S], f32, tag="offd32")
            nc.sync.dma_start(offd32[:], offd32_d[:])
            offd16 = cpool.tile([S, S], f16, tag="offd16")
            nc.sync.dma_start(offd16[:], offd16_d[:])

            sspanT = [None] * B_SH
            maskT32 = [None] * B_SH
            w16 = [None] * B_SH
            qz = [None] * B_SH
            spt = [[None] * S for _ in range(B_SH)]

            # Per-batch prep: W mask, transposed s_span / mask, initial qz.
            for b in range(B_SH):
                m8 = wpool.tile([S, S], u8, tag="m8")
                nc.sync.dma_start(m8[:], mask_d[b])
                m32 = wpool.tile([S, S], f32, tag="m32")
                nc.vector.tensor_copy(m32[:], m8[:])

                ssn = wpool.tile([S, S], f32, tag="ssn")
                nc.sync.dma_start(ssn[:], sspan_d[b])

                pt = pst_pool.tile([S, S], f32, tag="pst")
                nc.tensor.transpose(pt[:], m32[:], ident[:])
                maskT32[b] = ppool.tile([S, S], f32, name=f"maskT_{b}", tag=f"maskT_{b}")
                nc.vector.tensor_copy(maskT32[b][:], pt[:])

                pt2 = pst_pool.tile([S, S], f32, tag="pst")
                nc.tensor.transpose(pt2[:], ssn[:], ident[:])
                sspanT[b] = ppool.tile([S, S], f32, name=f"sspanT_{b}", tag=f"sspanT_{b}")
                nc.vector.tensor_copy(sspanT[b][:], pt2[:])

                wtmp = wpool.tile([S, S], f32, tag="wtmp")
                nc.vector.tensor_max(wtmp[:], m32[:], maskT32[b][:])
                w16[b] = ppool.tile([S, S], f16, name=f"w16_{b}", tag=f"w16_{b}")
                nc.vector.tensor_mul(w16[b][:], wtmp[:], offd32[:])

                # qz0 = sigmoid(s_span^T) with zeroed diagonal, fp16.
                qs = qz_pool.tile([S, S], f16, tag="qz")
                nc.scalar.activation(qs[:], sspanT[b][:],
                                     mybir.ActivationFunctionType.Sigmoid)
                qz0 = qz_pool.tile([S, S], f16, tag="qz")
                nc.vector.tensor_mul(qz0[:], qs[:], offd16[:])
                qz[b] = qz0

            # Stream s_pair: cast-load slab, mask, transpose per tile.
            for b in range(B_SH):
                for g in range(S // GRP):
                    nat = nat_pool.tile([S, GRP, S], f16, tag="nat")
                    src = spair_d[b, g * GRP:(g + 1) * GRP].rearrange(
                        "i j k -> j i k")
                    nc.gpsimd.dma_start(nat[:], src)
                    msk = msk_pool.tile([S, GRP, S], f16, tag="msk")
                    for t in range(GRP):
                        i = g * GRP + t
                        nc.vector.tensor_mul(msk[:, t, :], nat[:, t, :],
                                             w16[b][:])
                        st = spt_pool.tile([S, S], f16, tag="spt")
                        nc.sync.dma_start(st[:], msk[:, t, :], transpose=True)
                        spt[b][i] = st

            # MFVI iterations (interleave batches so PE ping-pongs).
            for t in range(MAX_ITER):
                for b in range(B_SH):
                    ps = ps_pool.tile([S, S], f32, tag="ps")
                    for i in range(S):
                        nc.tensor.matmul(ps[:, i:i + 1], spt[b][i][:],
                                         qz[b][:, i:i + 1],
                                         start=True, stop=True)
                    tmp = wpool.tile([S, S], f32, tag="tmp")
                    nc.vector.tensor_mul(tmp[:], ps[:], maskT32[b][:])
                    qacc = wpool.tile([S, S], f32, tag="qacc")
                    nc.vector.tensor_add(qacc[:], tmp[:], sspanT[b][:])
                    if t < MAX_ITER - 1:
                        qs = qz_pool.tile([S, S], f16, tag="qz")
                        nc.scalar.activation(
                            qs[:], qacc[:],
                            mybir.ActivationFunctionType.Sigmoid)
                        qn = qz_pool.tile([S, S], f16, tag="qz")
                        nc.vector.tensor_mul(qn[:], qs[:], offd16[:])
                        qz[b] = qn
                    else:
                        o32 = wpool.tile([S, S], f32, tag="o32")
                        nc.scalar.activation(
                            o32[:], qacc[:],
                            mybir.ActivationFunctionType.Sigmoid)
                        po = pst_pool.tile([S, S], f32, tag="pst")
                        nc.tensor.transpose(po[:], o32[:], ident[:])
                        onat = wpool.tile([S, S], f32, tag="onat")
                        nc.vector.tensor_copy(onat[:], po[:])
                        nc.sync.dma_start(out_d[b], onat[:])

    nc.compile()
    return nc


def _get_nc():
    global _compiled_nc
    if _compiled_nc is None:
        _compiled_nc = build_nc()
    return _compiled_nc


def make_in_maps(s_span, s_pair, mask):
    mask_u8 = np.ascontiguousarray(mask).view(np.uint8)
    eye = np.eye(S, dtype=np.float32)
    offd = (1.0 - eye).astype(np.float32)
    in_maps = []
    for c in range(N_CORES):
        sl = slice(c * B_SH, (c + 1) * B_SH)
        in_maps.append({
            "s_span": np.ascontiguousarray(s_span[sl], dtype=np.float32),
            "s_pair": np.ascontiguousarray(s_pair[sl], dtype=np.float32),
            "mask": np.ascontiguousarray(mask_u8[sl]),
            "ident": eye,
            "offd32": offd,
            "offd16": offd.astype(np.float16),
        })
    return in_maps


def kernel(s_span, s_pair, mask):
    nc = _get_nc()
    in_maps = make_in_maps(np.asarray(s_span), np.asarray(s_pair),
                           np.asarray(mask))
    res = run_bass_kernel_spmd(nc, in_maps, core_ids=list(range(N_CORES)))
    return np.concatenate([res.results[c]["out"] for c in range(N_CORES)],
                          axis=0)
